# revision 1
# baseline (speedup 1.0000x reference)
"""NVFP4 fake-quantized linear layer on 8 Trainium2 NeuronCores.

Computes: y = x @ dequant(nvfp4_quantize(weight)).T + bias
  x [8192, 4096] f32, weight [4096, 4096] f32, bias [4096] f32.

Strategy (tensor-parallel, per the row-wise sharding of weight):
  - weight rows (out_features) sharded 512/core; each core runs the
    per-(row,block-of-32) MSE scale search + final quantization in fp32
    (bit-faithful to the reference, incl. fp8-e4m3 scale rounding emulated
    in fp32 arithmetic), producing w_dq^T in bf16 (exact: products of a
    3-bit codebook value and a 4-bit fp8 scale fit in bf16's 8-bit
    significand).
  - x row-sliced 1024/core for a cooperative cast to bf16; slices are
    AllGathered so every core holds the full x in bf16, which the matmul
    phase streams back through the DMA transpose engine as [K,M] tiles.
  - Matmul: out_tile[n=128, m=512] accumulated over 32 K-chunks with
    w_dq^T chunks as the stationary operand; bias is added during the
    PSUM->SBUF copy on the scalar engine; each core writes its y^T shard
    [512, 8192] and the host concatenates + transposes.
"""

import sys

sys.path.insert(0, "/opt/trn_rl_repo")

from contextlib import ExitStack

import numpy as np

import concourse.bass as bass
import concourse.bacc as bacc
import concourse.tile as tile
from concourse import mybir
from concourse.bass_utils import run_bass_kernel_spmd

A = mybir.AluOpType
AF = mybir.ActivationFunctionType
F32 = mybir.dt.float32
BF16 = mybir.dt.bfloat16

NCORES = 8
M, K, N = 8192, 4096, 4096
MSH = M // NCORES          # 1024 x rows per core
NSH = N // NCORES          # 512 weight rows per core
NG = NSH // 128            # 4 row groups per core
KC = K // 128              # 32 contraction chunks
MGROUPS = M // 512         # 16 output column groups
MT = MSH // 128            # 8 x-slice tiles per core
KB = K // 32               # 128 blocks per weight row

RATIOS = [float(r) for r in np.linspace(0.7, 1.0, 10)]
MAGIC = 12582912.0         # 1.5 * 2**23 : RNE-to-integer magic constant
INF = float("inf")         # only used via memset (packs bits; JSON-safe)
I32 = mybir.dt.int32
EXP_MASK = 0x7F800000      # fp32 exponent field mask (int immediate)
TWO_BITS = 0x40000000      # bits of 2.0f; int max == float max for positives
# fp8 e4m3 rounding grid: step = max(2^-9, exp2floor(x) * 2^-3)
MAGIC8_HI = MAGIC / 8.0
MAGIC8_LO = MAGIC / 512.0


def build_nc() -> bass.Bass:
    nc = bacc.Bacc("TRN2", num_devices=NCORES)

    xs = nc.declare_dram_parameter("xs", [MSH, K], F32, isOutput=False)
    w = nc.declare_dram_parameter("w", [NSH, K], F32, isOutput=False)
    bias = nc.declare_dram_parameter("bias", [NSH, 1], F32, isOutput=False)
    yT = nc.declare_dram_parameter("yT", [NSH, M], F32, isOutput=True)

    with tile.TileContext(nc) as tc, ExitStack() as ctx:
        dram = ctx.enter_context(tc.tile_pool(name="dram", bufs=1, space="DRAM"))
        xbl = dram.tile([MSH, K], BF16, name="x_bf16_local")
        xbf = dram.tile([NCORES, MSH, K], BF16, addr_space="Shared", name="x_bf16_full")

        big = ctx.enter_context(tc.tile_pool(name="big", bufs=1))
        sm = ctx.enter_context(tc.tile_pool(name="small", bufs=1))
        psum = ctx.enter_context(tc.tile_pool(name="psum", bufs=1, space="PSUM"))
        xtp = ctx.enter_context(tc.tile_pool(name="xtp", bufs=3))
        ytp = ctx.enter_context(tc.tile_pool(name="ytp", bufs=2))

        # persistent w_dq^T, bf16 [128 k-partitions, 32 k-chunks, 512 n]
        wdqT = big.tile([128, KC, NSH], BF16, tag="wdqT", name="wdqT")

        ident = sm.tile([128, 128], BF16, tag="ident", name="ident")
        from concourse.masks import make_identity

        make_identity(nc, ident)

        # ---- phase X: cast this core's x slice to bf16 ----
        for mt in range(MT):
            xin = big.tile([128, K], F32, tag="s1", name="xin")
            nc.gpsimd.dma_start(out=xin, in_=xs[mt * 128 : (mt + 1) * 128, :])
            xb = big.tile([128, K], BF16, tag="sB", name="xb")
            nc.vector.tensor_copy(out=xb, in_=xin)
            nc.gpsimd.dma_start(out=xbl[mt * 128 : (mt + 1) * 128, :], in_=xb)

        # ---- phase G: allgather bf16 x slices ----
        nc.gpsimd.collective_compute(
            "AllGather",
            A.bypass,
            replica_groups=[list(range(NCORES))],
            ins=[xbl.opt()],
            outs=[xbf.opt()],
        )

        # ---- phase Q: quantize weight shard, 128 rows per group ----
        bias_sb = []
        for g in range(NG):
            bsl = sm.tile([128, 1], F32, tag=f"bias{g}", name=f"bias{g}")
            nc.gpsimd.dma_start(out=bsl, in_=bias[g * 128 : (g + 1) * 128, :])
            bias_sb.append(bsl)

        for g in range(NG):
            wt = big.tile([128, K], F32, tag="s3", bufs=2, name="wt")
            nc.gpsimd.dma_start(out=wt, in_=w[g * 128 : (g + 1) * 128, :])
            wt3 = wt.rearrange("p (b e) -> p b e", e=32)

            bmax = sm.tile([128, KB], F32, tag="bmax", name="bmax")
            nc.vector.tensor_reduce(
                out=bmax, in_=wt3, axis=mybir.AxisListType.X, op=A.max,
                apply_absolute_value=True,
            )
            nc.vector.tensor_scalar(out=bmax, in0=bmax, scalar1=1e-12, scalar2=None, op0=A.max)
            inv = sm.tile([128, KB], F32, tag="inv", name="inv")
            nc.vector.reciprocal(out=inv, in_=bmax)

            # b2s = w * 12 / bmax  (signed; |b2s| in [0, 12])
            b2s = big.tile([128, K], F32, tag="b2s", name="b2s")
            b2s3 = b2s.rearrange("p (b e) -> p b e", e=32)
            inv_b = inv.unsqueeze(2).broadcast_to([128, KB, 32])
            nc.vector.scalar_tensor_tensor(
                out=b2s3, in0=wt3, scalar=12.0, in1=inv_b, op0=A.mult, op1=A.mult,
            )

            best_e = sm.tile([128, KB], F32, tag="best_e", name="best_e")
            nc.vector.memset(best_e, INF)
            best_c = sm.tile([128, KB], F32, tag="best_c", name="best_c")
            nc.vector.memset(best_c, 0.0)
            best_r = sm.tile([128, KB], F32, tag="best_r", name="best_r")
            nc.vector.memset(best_r, 1.0)

            for ratio in RATIOS:
                c = float(np.float32(1.0) / np.float32(ratio))
                B2s = big.tile([128, K], F32, tag="sA", name="B2s")
                nc.vector.tensor_scalar(out=B2s, in0=b2s, scalar1=c, scalar2=None, op0=A.mult)
                # exponent bucket: t = max(exp2floor(|B2s|), 2)
                tta = big.tile([128, K], F32, tag="sB", name="tta")
                nc.vector.tensor_scalar(
                    out=tta.bitcast(I32), in0=B2s.bitcast(I32),
                    scalar1=EXP_MASK, scalar2=None, op0=A.bitwise_and,
                )
                # magic value ms = max(exp2floor(|B2s|), 2) * (MAGIC/2)
                msv = big.tile([128, K], F32, tag="sC", name="msv")
                nc.vector.tensor_scalar(
                    out=msv, in0=tta, scalar1=2.0, scalar2=MAGIC / 2.0, op0=A.max, op1=A.mult,
                )
                r_ = big.tile([128, K], F32, tag="sD", name="r")
                nc.vector.tensor_tensor(out=r_, in0=B2s, in1=msv, op=A.add)
                q2u = big.tile([128, K], F32, tag="sB", name="q2u")
                nc.vector.tensor_tensor(out=q2u, in0=r_, in1=msv, op=A.subtract)
                q2c = big.tile([128, K], F32, tag="sC", name="q2c")
                nc.vector.tensor_scalar(
                    out=q2c, in0=q2u, scalar1=12.0, scalar2=-12.0, op0=A.min, op1=A.max,
                )
                d_ = big.tile([128, K], F32, tag="sD", name="d")
                nc.vector.tensor_tensor(out=d_, in0=B2s, in1=q2c, op=A.subtract)
                dsq = big.tile([128, K], F32, tag="s1", name="dsq")
                nc.scalar.activation(out=dsq, in_=d_, func=AF.Square)
                e_ = sm.tile([128, KB], F32, tag="e", name="e")
                nc.vector.tensor_reduce(
                    out=e_, in_=dsq.rearrange("p (b e) -> p b e", e=32),
                    axis=mybir.AxisListType.X, op=A.add,
                )
                rr = float(np.float32(ratio) * np.float32(ratio))
                nc.vector.tensor_scalar(out=e_, in0=e_, scalar1=rr, scalar2=None, op0=A.mult)
                mask = sm.tile([128, KB], I32, tag="mask", name="mask")
                nc.vector.tensor_tensor(out=mask, in0=e_, in1=best_e, op=A.is_lt)
                nc.vector.tensor_tensor(out=best_e, in0=e_, in1=best_e, op=A.min)
                cconst = sm.tile([128, KB], F32, tag="cconst", name="cconst")
                nc.vector.memset(cconst, c)
                nc.vector.copy_predicated(out=best_c, mask=mask, data=cconst)
                rconst = sm.tile([128, KB], F32, tag="rconst", name="rconst")
                nc.vector.memset(rconst, float(np.float32(ratio)))
                nc.vector.copy_predicated(out=best_r, mask=mask, data=rconst)

            # scale factor sf = bmax * best_r / 6, rounded to fp8 e4m3 (RNE,
            # subnormal-aware) emulated in fp32, then halved (q = q2/2).
            sf = sm.tile([128, KB], F32, tag="sf", name="sf")
            nc.vector.scalar_tensor_tensor(
                out=sf, in0=bmax, scalar=1.0 / 6.0, in1=best_r, op0=A.mult, op1=A.mult,
            )
            eb8 = sm.tile([128, KB], F32, tag="eb8", name="eb8")
            nc.vector.tensor_scalar(
                out=eb8.bitcast(I32), in0=sf.bitcast(I32),
                scalar1=EXP_MASK, scalar2=None, op0=A.bitwise_and,
            )
            ms8 = sm.tile([128, KB], F32, tag="ms8", name="ms8")
            nc.vector.tensor_scalar(
                out=ms8, in0=eb8, scalar1=MAGIC8_HI, scalar2=MAGIC8_LO, op0=A.mult, op1=A.max,
            )
            nc.vector.tensor_tensor(out=sf, in0=sf, in1=ms8, op=A.add)
            nc.vector.tensor_tensor(out=sf, in0=sf, in1=ms8, op=A.subtract)
            nc.vector.tensor_scalar(out=sf, in0=sf, scalar1=0.5, scalar2=None, op0=A.mult)

            # final quantization with the chosen scale
            B2f = big.tile([128, K], F32, tag="sA", name="B2f")
            B2f3 = B2f.rearrange("p (b e) -> p b e", e=32)
            bc_b = best_c.unsqueeze(2).broadcast_to([128, KB, 32])
            nc.vector.tensor_tensor(out=B2f3, in0=b2s3, in1=bc_b, op=A.mult)
            tta = big.tile([128, K], F32, tag="sB", name="ttaf")
            nc.vector.tensor_scalar(
                out=tta.bitcast(I32), in0=B2f.bitcast(I32),
                scalar1=EXP_MASK, scalar2=None, op0=A.bitwise_and,
            )
            msv = big.tile([128, K], F32, tag="sC", name="msvf")
            nc.vector.tensor_scalar(
                out=msv, in0=tta, scalar1=2.0, scalar2=MAGIC / 2.0, op0=A.max, op1=A.mult,
            )
            r_ = big.tile([128, K], F32, tag="sD", name="rf")
            nc.vector.tensor_tensor(out=r_, in0=B2f, in1=msv, op=A.add)
            q2u = big.tile([128, K], F32, tag="sB", name="q2uf")
            nc.vector.tensor_tensor(out=q2u, in0=r_, in1=msv, op=A.subtract)
            q2c = big.tile([128, K], F32, tag="sC", name="q2cf")
            nc.vector.tensor_scalar(
                out=q2c, in0=q2u, scalar1=12.0, scalar2=-12.0, op0=A.min, op1=A.max,
            )
            wdq = big.tile([128, K], BF16, tag="wdq", bufs=2, name="wdq")
            sf_b = sf.unsqueeze(2).broadcast_to([128, KB, 32])
            nc.vector.tensor_tensor(
                out=wdq.rearrange("p (b e) -> p b e", e=32), in0=q2c.rearrange("p (b e) -> p b e", e=32),
                in1=sf_b, op=A.mult,
            )

            # transpose into wdqT[:, kc, g*128:(g+1)*128]
            for kc in range(KC):
                pt = psum.tile([128, 128], BF16, tag="ptr", bufs=2, name="pt")
                nc.tensor.transpose(pt, wdq[:, kc * 128 : (kc + 1) * 128], ident)
                nc.scalar.copy(
                    out=wdqT[:, kc, g * 128 : (g + 1) * 128], in_=pt,
                )

        # ---- phase M: matmul ----
        for mg in range(MGROUPS):
            c_src = mg // (MGROUPS // NCORES)
            m0 = (mg % (MGROUPS // NCORES)) * 512
            psums = [
                psum.tile([128, 512], F32, tag=f"pm{g}", name=f"pm{g}") for g in range(NG)
            ]
            for kc in range(KC):
                xt = xtp.tile([128, 512], BF16, tag="xt", name="xt")
                nc.sync.dma_start_transpose(
                    xt, xbf[c_src, m0 : m0 + 512, kc * 128 : (kc + 1) * 128]
                )
                for g in range(NG):
                    nc.tensor.matmul(
                        psums[g],
                        lhsT=wdqT[:, kc, g * 128 : (g + 1) * 128],
                        rhs=xt,
                        start=(kc == 0),
                        stop=(kc == KC - 1),
                    )
            for g in range(NG):
                ysb = ytp.tile([128, 512], F32, tag="ysb", name="ysb")
                nc.scalar.add(out=ysb, in_=psums[g], add=bias_sb[g])
                nc.sync.dma_start(
                    out=yT[g * 128 : (g + 1) * 128, mg * 512 : (mg + 1) * 512], in_=ysb
                )

    nc.compile()
    return nc


_NC_CACHE = None


def kernel(x: np.ndarray, weight: np.ndarray, bias: np.ndarray) -> np.ndarray:
    global _NC_CACHE
    if _NC_CACHE is None:
        _NC_CACHE = build_nc()
    nc = _NC_CACHE

    x = np.ascontiguousarray(x, dtype=np.float32)
    weight = np.ascontiguousarray(weight, dtype=np.float32)
    bias = np.ascontiguousarray(bias, dtype=np.float32)

    in_maps = []
    for c in range(NCORES):
        in_maps.append(
            {
                "xs": x[c * MSH : (c + 1) * MSH],
                "w": weight[c * NSH : (c + 1) * NSH],
                "bias": bias[c * NSH : (c + 1) * NSH].reshape(NSH, 1),
            }
        )
    res = run_bass_kernel_spmd(nc, in_maps, list(range(NCORES)))
    yT = np.concatenate([res.results[c]["yT"] for c in range(NCORES)], axis=0)
    return np.ascontiguousarray(yT.T)


def profile_once(x, weight, bias):
    global _NC_CACHE
    if _NC_CACHE is None:
        _NC_CACHE = build_nc()
    nc = _NC_CACHE
    x = np.ascontiguousarray(x, dtype=np.float32)
    weight = np.ascontiguousarray(weight, dtype=np.float32)
    bias = np.ascontiguousarray(bias, dtype=np.float32)
    in_maps = []
    for c in range(NCORES):
        in_maps.append(
            {
                "xs": x[c * MSH : (c + 1) * MSH],
                "w": weight[c * NSH : (c + 1) * NSH],
                "bias": bias[c * NSH : (c + 1) * NSH].reshape(NSH, 1),
            }
        )
    res = run_bass_kernel_spmd(
        nc, in_maps, list(range(NCORES)), trace=True, tmpdir="/tmp/nvfp4_trace"
    )
    print("exec_time_ns:", res.exec_time_ns, "mean:", res.mean_exec_time_ns,
          "max_core:", res.max_exec_time_core_id)
    return res.exec_time_ns

